# revision 14
# baseline (speedup 1.0000x reference)
"""Bass/Tile TRN2 kernel for nn_CropDrones.

Op: per-sample, find bbox of a binary window mask (channel 3 of input1),
crop rows [r0, r1) x cols [c0, c1) of the 3 image channels, and paste the
crop centered into a 256x256 zero canvas.

Sharding: pure data parallel - batch 32 split as 8 cores x 4 samples.

Device strategy (per sample, all control flow static; data dependence
flows only through values):
  1. Load the 512x512 mask as [128p, 4n, 512w]. Row sums via 4 Activation
     Identity+accum ops (srow[p,n] = sum over w; equals sw+1 inside the
     window, 0 outside). Col stats on DVE: tp2/colp pairwise maxes give
     colp[p,w] = colany[w]*rowanyp[p]; the producing scalar_tensor_tensor
     emits accum_out = s~ = (sw+1)*rowanyp free of charge, and one more
     st against iota_w gives the first moment m~ = sum(w*colany)*rowanyp.
  2. One gpsimd partition_all_reduce(max) over [r1cand, r0revcand, s~, m~]
     broadcasts exact bbox stats to all partitions (r candidates from the
     0/1 rowany weighted-max trick; s~/m~ scale with rowanyp so the max
     over partitions is the clean value).
  3. c0+c1 = 2*m/s recovered exactly via DVE reciprocal + f32->i32
     round-to-nearest (error ~1e-4 << 0.5); top/left via i32 shift-right.
  4. Row gather fused with the fine column shift: per channel c, ONE
     element-granular indirect DMA (src viewed as [N,1], coef=1): index
     of partition p = base + clamp(rt+2p,0,511)*512 + (c0-left), each
     copying 768 contiguous floats = output rows (2p, 2p+1) at offsets
     0..256 and 512..768. No ap_gather, no full-row overfetch.
  5. Masks: mx (col validity) on DVE, mk[k] = my[k]*mx built on the
     Activation engine; fin = gat*mk via 3 tensor_tensor ops; one DMA
     stores [128, (c k x)] as y[i, c, 2p+k, x].

Verified bit-exact vs the jax reference (relative error 0.0).
"""

import numpy as np

import concourse.bass as bass
import concourse.bacc as bacc
import concourse.bass_isa as bass_isa
import concourse.mybir as mybir
from concourse.bass import IndirectOffsetOnAxis
from concourse.bass_utils import run_bass_kernel_spmd
from concourse.tile import TileContext

# Problem shapes (hardcoded; kernel.py must be self-contained).
B, C, H, W = 32, 3, 512, 512
CH_IN = 4          # image channels + mask channel
S = 256            # output side
N_CORES = 8
BPC = B // N_CORES  # samples per core
P = 128
NT = H // P        # mask row tiles
NEL = BPC * CH_IN * H * W  # flat element count of x
GL = 2 * W - S     # gathered run per index: rows (2p, 2p+1) -> 768 floats

f32 = mybir.dt.float32
i32 = mybir.dt.int32
Alu = mybir.AluOpType
Ax = mybir.AxisListType
Act = mybir.ActivationFunctionType


_C_WIDTHS = {
    "c_iota_w": W,     # [128, 512] 0..511 per row
    "c_iota_r": NT,    # [128, 4] r = n*128+p
    "c_rev_r": NT,     # 511 - r
    "c_y2": 2,         # output rows per partition: 2p, 2p+1
    "c_band": 1,       # p (coarse band index)
    "c_bandrev": 1,    # 31-p
    "c_lo16": 1,       # p < 16
    "c_hi16": 1,       # p >= 16
    "c_pp": 1,         # p
    "c_2p": 1,         # 2p
    "c_coff": C,       # c * H * W
}
_C_OFFS = {}
_off = 0
for _k, _w in _C_WIDTHS.items():
    _C_OFFS[_k] = _off
    _off += _w
C_TOTAL = _off


def _consts() -> dict[str, np.ndarray]:
    p = np.arange(P)
    iota_w = np.broadcast_to(np.arange(W, dtype=np.float32), (P, W)).copy()
    iota_r = (p[:, None] + P * np.arange(NT)[None, :]).astype(np.float32)
    rev_r = (H - 1.0) - iota_r
    y2 = (2 * p[:, None] + np.arange(2)[None, :]).astype(np.float32)
    band = p[:, None].astype(np.float32)
    bandrev = (31.0 - p[:, None]).astype(np.float32)
    lo16 = (p[:, None] < 16).astype(np.float32)
    hi16 = ((p[:, None] >= 16) & (p[:, None] < 32)).astype(np.float32)
    pp = p[:, None].astype(np.float32)
    c2p = (2.0 * p[:, None]).astype(np.float32)
    coff = np.broadcast_to(
        (np.arange(C) * H * W).astype(np.float32), (P, C)
    ).copy()
    vals = {
        "c_iota_w": iota_w,
        "c_iota_r": iota_r,
        "c_rev_r": rev_r,
        "c_y2": y2,
        "c_band": band,
        "c_bandrev": bandrev,
        "c_lo16": lo16,
        "c_hi16": hi16,
        "c_pp": pp,
        "c_2p": c2p,
        "c_coff": coff,
    }
    packed = np.zeros((P, C_TOTAL), dtype=np.float32)
    for kk, vv in vals.items():
        packed[:, _C_OFFS[kk] : _C_OFFS[kk] + _C_WIDTHS[kk]] = vv
    return {"c_all": packed}


def _build() -> bass.Bass:
    nc = bacc.Bacc("TRN2")
    x = nc.dram_tensor("x", [BPC, CH_IN, H, W], f32, kind="ExternalInput")
    y = nc.dram_tensor("y", [BPC, C, S, S], f32, kind="ExternalOutput")
    c_all = nc.dram_tensor("c_all", [P, C_TOTAL], f32, kind="ExternalInput")

    with TileContext(nc) as tc:
        with (
            tc.tile_pool(name="consts", bufs=1) as cpool,
            tc.tile_pool(name="work", bufs=4) as wp,
        ):
            call_t = cpool.tile([P, C_TOTAL], f32, tag="c_all", name="c_all_t")
            nc.sync.dma_start(call_t[:], c_all[:])
            ct = {
                k: call_t[:, _C_OFFS[k] : _C_OFFS[k] + _C_WIDTHS[k]]
                for k in _C_WIDTHS
            }
            # warm the Activation function table before sample work arrives
            warm = cpool.tile([P, 1], f32, tag="warm", name="warm")
            nc.scalar.activation(warm[:], call_t[:, 0:1], Act.Identity, scale=1.0)

            ts = nc.vector.tensor_scalar
            tsp = nc.gpsimd.tensor_scalar
            st = nc.vector.scalar_tensor_tensor
            tt = nc.vector.tensor_tensor
            ttp = nc.gpsimd.tensor_tensor
            red = nc.vector.tensor_reduce
            cpy = nc.vector.tensor_copy
            cpyp = nc.gpsimd.tensor_copy
            act = nc.scalar.activation

            for i in range(BPC):
                # ---- 1. coarse mask: rows 0,16,..,496 on partitions 0..31 ----
                mc = wp.tile([P, W], f32, tag="mc", name=f"mc{i}")
                xm = x[i, CH_IN - 1]
                nc.sync.dma_start(
                    mc[0:32, :],
                    bass.AP(xm.tensor, xm.offset, [[16 * W, 32], [1, W]]),
                )

                hp = tc.high_priority(offset=300)
                hp.__enter__()
                # per coarse row p: s~ = rowsum (sw+1 if in-window), m~ = moment
                cand = wp.tile([P, 4], f32, tag="cand", name=f"cand{i}")
                junkd = wp.tile([P, W], f32, tag="junkd", name=f"jd{i}")
                nc.vector.memset(cand[:], 0.0)
                red(cand[0:32, 2:3], mc[0:32, :], axis=Ax.X, op=Alu.add)  # s~
                st(junkd[0:32, :], mc[0:32, :], 1.0, ct["c_iota_w"][0:32, :],
                   op0=Alu.mult, op1=Alu.mult, accum_out=cand[0:32, 3:4])  # m~
                # band-any -> [k2cand, k1revcand]
                ra = wp.tile([P, 3], f32, tag="ra", name=f"ra{i}")
                ts(ra[0:32, 0:1], cand[0:32, 2:3], 0.0, None, op0=Alu.is_gt)
                tt(cand[0:32, 0:1], ra[0:32, 0:1], ct["c_band"][0:32, :],
                   op=Alu.mult)                           # k*any
                tt(cand[0:32, 1:2], ra[0:32, 0:1], ct["c_bandrev"][0:32, :],
                   op=Alu.mult)                           # (31-k)*any
                ar = wp.tile([P, 4], f32, tag="ar", name=f"ar{i}")
                nc.gpsimd.partition_all_reduce(
                    ar[:], cand[:], channels=P, reduce_op=bass_isa.ReduceOp.max
                )  # ar = [k2, 31-k1, s, m]

                # ---- 1b. refine: rows 16k1-15..16k1 and 16k2..16k2+15 ----
                # row(p) = clamp(lo16*(481-16*(31-k1)) + hi16*(16*k2-16) + p, 0)
                rr = wp.tile([P, 4], f32, tag="rr", name=f"rr{i}")
                ri2 = wp.tile([P, 1], i32, tag="ri2", name=f"ri2{i}")
                tsp(rr[:, 0:1], ar[:, 1:2], -16.0, 481.0, op0=Alu.mult, op1=Alu.add)
                tsp(rr[:, 1:2], ar[:, 0:1], 16.0, -16.0, op0=Alu.mult, op1=Alu.add)
                tsp(rr[:, 0:1], ct["c_lo16"], rr[:, 0:1], None, op0=Alu.mult)
                tsp(rr[:, 1:2], ct["c_hi16"], rr[:, 1:2], None, op0=Alu.mult)
                ttp(rr[:, 2:3], rr[:, 0:1], rr[:, 1:2], op=Alu.add)
                ttp(rr[:, 2:3], rr[:, 2:3], ct["c_pp"], op=Alu.add)
                tsp(rr[:, 2:3], rr[:, 2:3], 0.0, None, op0=Alu.max)  # row
                tsp(rr[:, 3:4], rr[:, 2:3], float(W),
                    float((CH_IN - 1) * H * W + i * CH_IN * H * W),
                    op0=Alu.mult, op1=Alu.add)            # flat idx
                cpyp(ri2[:], rr[:, 3:4])
                rf = wp.tile([P, W], f32, tag="rf", name=f"rf{i}")
                binst = nc.gpsimd.indirect_dma_start(
                    out=rf[0:32, :],
                    out_offset=None,
                    in_=bass.AP(x[:].tensor, 0, [[1, NEL - W + 1], [1, W]]),
                    in_offset=IndirectOffsetOnAxis(ap=ri2[0:32, :], axis=0),
                )
                a0 = binst.ins.ins[0]
                d0 = a0.dynamic_ap_info
                a0.dynamic_ap_info = mybir.DynamicAccessPatternInfo(
                    c=d0.c, actual_ap=d0.actual_ap,
                    indirect_dim_max_index=d0.indirect_dim_max_index,
                    offset_expr=[mybir.DynamicAccessPatternOffsetExpr(
                        coef=1, aff_expr=d0.offset_expr[0].aff_expr)])

                # refine row-any (full width) -> exact r0/r1 weighted maxes
                cand2 = wp.tile([P, 2], f32, tag="cand2", name=f"c2{i}")
                nc.vector.memset(cand2[:], 0.0)
                red(ra[0:32, 1:2], rf[0:32, :], axis=Ax.X, op=Alu.max)  # any
                # weights: p<16: (511-row)*any ; p>=16: row*any
                ts(ra[0:32, 2:3], rr[0:32, 2:3], -1.0, 511.0,
                   op0=Alu.mult, op1=Alu.add)
                tt(ra[0:32, 2:3], ra[0:32, 2:3], ct["c_lo16"][0:32, :],
                   op=Alu.mult)
                tt(cand2[0:32, 0:1], ra[0:32, 1:2], ra[0:32, 2:3], op=Alu.mult)
                ts(ra[0:32, 2:3], rr[0:32, 2:3], ct["c_hi16"][0:32, :], None,
                   op0=Alu.mult)
                tt(cand2[0:32, 1:2], ra[0:32, 1:2], ra[0:32, 2:3], op=Alu.mult)
                ar2 = wp.tile([P, 2], f32, tag="ar2", name=f"ar2{i}")
                nc.gpsimd.partition_all_reduce(
                    ar2[:], cand2[:], channels=P, reduce_op=bass_isa.ReduceOp.max
                )  # ar2 = [511-r0, r1]

                # ---- 4. scalar chain: recip on DVE, rest on Pool ----
                # ar = [r1, 511-r0, s~=sw+1, m~]
                # sc: 0 s1=r1+(511-r0), 1 rs, 2 q=2mu, 3 kf, 4 t_top,
                #     5 t_left, 6 top, 7 left, 8 c0, 9 rt, 10 d, 11 tph, 12 lw
                sc = wp.tile([P, 13], f32, tag="sc", name=f"sc{i}")
                sci = wp.tile([P, 3], i32, tag="sci", name=f"sci{i}")
                nc.vector.reciprocal(sc[:, 1:2], ar[:, 2:3])
                tsp(sc[:, 0:1], ar2[:, 1:2], ar2[:, 0:1], None, op0=Alu.add)
                tsp(sc[:, 2:3], ar[:, 3:4], sc[:, 1:2], 2.0,
                    op0=Alu.mult, op1=Alu.mult)           # 2*mu approx
                cpyp(sci[:, 0:1], sc[:, 2:3])             # round -> k=c0+c1
                cpyp(sc[:, 3:4], sci[:, 0:1])             # kf
                # floor((256-side)/2) via round(x*0.5-0.25):
                # t_top = (767-s1)/2-0.25; t_left = (257-s)/2-0.25
                tsp(sc[:, 4:5], sc[:, 0:1], -0.5, 383.25, op0=Alu.mult, op1=Alu.add)
                tsp(sc[:, 5:6], ar[:, 2:3], -0.5, 128.25, op0=Alu.mult, op1=Alu.add)
                cpyp(sci[:, 1:3], sc[:, 4:6])             # round-to-nearest
                cpyp(sc[:, 6:8], sci[:, 1:3])             # top, left
                tsp(sc[:, 8:9], sc[:, 3:4], ar[:, 2:3], 1.0,
                    op0=Alu.subtract, op1=Alu.add)        # 2c0 = k - s + 1
                tsp(sc[:, 8:9], sc[:, 8:9], 0.5, None, op0=Alu.mult)  # c0
                tsp(sc[:, 9:10], ar2[:, 0:1], -1.0, 511.0,
                    op0=Alu.mult, op1=Alu.add)            # r0 (reuse col 9)
                ttp(sc[:, 9:10], sc[:, 9:10], sc[:, 6:7], op=Alu.subtract)  # rt
                ttp(sc[:, 10:11], sc[:, 8:9], sc[:, 7:8], op=Alu.subtract)  # d
                tsp(sc[:, 11:12], sc[:, 6:7], sc[:, 0:1], -511.0,
                    op0=Alu.add, op1=Alu.add)             # tph = top+sh
                tsp(sc[:, 12:13], sc[:, 7:8], ar[:, 2:3], -1.0,
                    op0=Alu.add, op1=Alu.add)             # lw = left+sw

                # ---- 5. gather indices + 3 indirect DMAs ----
                iy = wp.tile([P, 1], f32, tag="iy", name=f"iy{i}")
                ix = wp.tile([P, C], f32, tag="ix", name=f"ix{i}")
                ri = wp.tile([P, C], i32, tag="ri", name=f"ri{i}")
                tsp(iy[:], ct["c_2p"], sc[:, 9:10], None, op0=Alu.add)
                tsp(iy[:], iy[:], 0.0, float(H - 1), op0=Alu.max, op1=Alu.min)
                tsp(iy[:], iy[:], float(W), None, op0=Alu.mult)
                tsp(ix[:], ct["c_coff"], iy[:], float(i * CH_IN * H * W),
                    op0=Alu.add, op1=Alu.add)             # coff + y*512 + base
                tsp(ix[:], ix[:], sc[:, 10:11], 0.0,
                    op0=Alu.add, op1=Alu.max)             # + d, clamp >= 0
                cpyp(ri[:], ix[:])
                hp.__exit__(None, None, None)

                gat = wp.tile([P, C, GL], f32, tag="gat", name=f"gat{i}")
                for c in range(C):
                    # src viewed as overlapping GL-wide rows so descgen emits
                    # 128 x 3KB descriptors; coef patched to 1 for
                    # element-granular starts (row r, col d in one index).
                    binst = nc.gpsimd.indirect_dma_start(
                        out=gat[:, c, :],
                        out_offset=None,
                        in_=bass.AP(x[:].tensor, 0,
                                    [[1, NEL - GL + 1], [1, GL]]),
                        in_offset=IndirectOffsetOnAxis(ap=ri[:, c : c + 1], axis=0),
                    )
                    a0 = binst.ins.ins[0]
                    d0 = a0.dynamic_ap_info
                    a0.dynamic_ap_info = mybir.DynamicAccessPatternInfo(
                        c=d0.c, actual_ap=d0.actual_ap,
                        indirect_dim_max_index=d0.indirect_dim_max_index,
                        offset_expr=[mybir.DynamicAccessPatternOffsetExpr(
                            coef=1, aff_expr=d0.offset_expr[0].aff_expr)])

                # ---- 6. masks ----
                mya = wp.tile([P, 2], f32, tag="mya", name=f"mya{i}")
                myb = wp.tile([P, 2], f32, tag="myb", name=f"myb{i}")
                tsp(mya[:], ct["c_y2"], sc[:, 6:7], None, op0=Alu.is_ge)
                tsp(myb[:], ct["c_y2"], sc[:, 11:12], None, op0=Alu.is_lt)
                ttp(mya[:], mya[:], myb[:], op=Alu.mult)  # my [128,2]
                mxa = wp.tile([P, S], f32, tag="mxa", name=f"mxa{i}")
                mxb = wp.tile([P, S], f32, tag="mxb", name=f"mxb{i}")
                ts(mxa[:], ct["c_iota_w"][:, :S], sc[:, 7:8], None, op0=Alu.is_ge)
                ts(mxb[:], ct["c_iota_w"][:, :S], sc[:, 12:13], None, op0=Alu.is_lt)
                tt(mxa[:], mxa[:], mxb[:], op=Alu.mult)   # mx [128,256]
                mk = wp.tile([P, 2, S], f32, tag="mk", name=f"mk{i}")
                for k in range(2):
                    act(mk[:, k, :], mxa[:], Act.Identity,
                        scale=mya[:, k : k + 1])

                # ---- 7. apply masks + store ----
                fin = wp.tile([P, C, 2, S], f32, tag="fin", name=f"fin{i}")
                yv = y[i]
                for c in range(C):
                    g = gat[:, c, :]
                    gv = bass.AP(g.tensor, g.offset,
                                 [list(g.ap[0]), [W, 2], [1, S]])
                    tt(fin[:, c], gv, mk[:], op=Alu.mult)
                    # y[i, c, 2p+k, x] <- fin[p, c, k, x]; (k x) merge to 512
                    ydst = bass.AP(yv.tensor, yv.offset + c * S * S,
                                   [[2 * S, P], [1, 2 * S]])
                    nc.sync.dma_start(ydst, fin[:, c].rearrange("p k x -> p (k x)"))
    nc.finalize()
    return nc


_CACHE: dict[str, object] = {}


def kernel(input1: np.ndarray, input2: np.ndarray, **_: np.ndarray) -> np.ndarray:
    input1 = np.ascontiguousarray(np.asarray(input1, dtype=np.float32))
    if "nc" not in _CACHE:
        _CACHE["nc"] = _build()
        _CACHE["consts"] = _consts()
    nc = _CACHE["nc"]
    consts = _CACHE["consts"]
    in_maps = [
        {"x": np.ascontiguousarray(input1[k * BPC : (k + 1) * BPC]), **consts}
        for k in range(N_CORES)
    ]
    res = run_bass_kernel_spmd(nc, in_maps, core_ids=list(range(N_CORES)))
    out = np.concatenate([r["y"] for r in res.results], axis=0)
    return out.astype(np.float32)


if __name__ == "__main__":
    rng = np.random.default_rng(1)
    x = rng.standard_normal((B, CH_IN, H, W), dtype=np.float32)
    print(kernel(x, np.zeros((B, C, S, S), np.float32)).shape)


# revision 15
# speedup vs baseline: 1.1066x; 1.1066x over previous
"""Bass/Tile TRN2 kernel for nn_CropDrones.

Op: per-sample, find bbox of a binary window mask (channel 3 of input1),
crop rows [r0, r1) x cols [c0, c1) of the 3 image channels, and paste the
crop centered into a 256x256 zero canvas.

Sharding: pure data parallel - batch 32 split as 8 cores x 4 samples.

Device strategy (per sample, all control flow static; data dependence
flows only through values):
  1. Load the 512x512 mask as [128p, 4n, 512w]. Row sums via 4 Activation
     Identity+accum ops (srow[p,n] = sum over w; equals sw+1 inside the
     window, 0 outside). Col stats on DVE: tp2/colp pairwise maxes give
     colp[p,w] = colany[w]*rowanyp[p]; the producing scalar_tensor_tensor
     emits accum_out = s~ = (sw+1)*rowanyp free of charge, and one more
     st against iota_w gives the first moment m~ = sum(w*colany)*rowanyp.
  2. One gpsimd partition_all_reduce(max) over [r1cand, r0revcand, s~, m~]
     broadcasts exact bbox stats to all partitions (r candidates from the
     0/1 rowany weighted-max trick; s~/m~ scale with rowanyp so the max
     over partitions is the clean value).
  3. c0+c1 = 2*m/s recovered exactly via DVE reciprocal + f32->i32
     round-to-nearest (error ~1e-4 << 0.5); top/left via i32 shift-right.
  4. Row gather fused with the fine column shift: per channel c, ONE
     element-granular indirect DMA (src viewed as [N,1], coef=1): index
     of partition p = base + clamp(rt+2p,0,511)*512 + (c0-left), each
     copying 768 contiguous floats = output rows (2p, 2p+1) at offsets
     0..256 and 512..768. No ap_gather, no full-row overfetch.
  5. Masks: mx (col validity) on DVE, mk[k] = my[k]*mx built on the
     Activation engine; fin = gat*mk via 3 tensor_tensor ops; one DMA
     stores [128, (c k x)] as y[i, c, 2p+k, x].

Verified bit-exact vs the jax reference (relative error 0.0).
"""

import numpy as np

import concourse.bass as bass
import concourse.bacc as bacc
import concourse.bass_isa as bass_isa
import concourse.mybir as mybir
from concourse.bass import IndirectOffsetOnAxis
from concourse.bass_utils import run_bass_kernel_spmd
from concourse.tile import TileContext

# Problem shapes (hardcoded; kernel.py must be self-contained).
B, C, H, W = 32, 3, 512, 512
CH_IN = 4          # image channels + mask channel
S = 256            # output side
N_CORES = 8
BPC = B // N_CORES  # samples per core
P = 128
NT = H // P        # mask row tiles
NEL = BPC * CH_IN * H * W  # flat element count of x
GL = 2 * W - S     # gathered run per index: rows (2p, 2p+1) -> 768 floats

f32 = mybir.dt.float32
i32 = mybir.dt.int32
Alu = mybir.AluOpType
Ax = mybir.AxisListType
Act = mybir.ActivationFunctionType


_C_WIDTHS = {
    "c_iota_w": W,     # [128, 512] 0..511 per row
    "c_iota_r": NT,    # [128, 4] r = n*128+p
    "c_rev_r": NT,     # 511 - r
    "c_y2": 2,         # output rows per partition: 2p, 2p+1
    "c_band": 1,       # p (coarse band index)
    "c_bandrev": 1,    # 31-p
    "c_lo16": 1,       # p < 16
    "c_hi16": 1,       # p >= 16
    "c_pp": 1,         # p
    "c_2p": 1,         # 2p
    "c_coff": C,       # c * H * W
}
_C_OFFS = {}
_off = 0
for _k, _w in _C_WIDTHS.items():
    _C_OFFS[_k] = _off
    _off += _w
C_TOTAL = _off


def _consts() -> dict[str, np.ndarray]:
    p = np.arange(P)
    iota_w = np.broadcast_to(np.arange(W, dtype=np.float32), (P, W)).copy()
    iota_r = (p[:, None] + P * np.arange(NT)[None, :]).astype(np.float32)
    rev_r = (H - 1.0) - iota_r
    y2 = (2 * p[:, None] + np.arange(2)[None, :]).astype(np.float32)
    band = p[:, None].astype(np.float32)
    bandrev = (31.0 - p[:, None]).astype(np.float32)
    lo16 = (p[:, None] < 16).astype(np.float32)
    hi16 = ((p[:, None] >= 16) & (p[:, None] < 32)).astype(np.float32)
    pp = p[:, None].astype(np.float32)
    c2p = (2.0 * p[:, None]).astype(np.float32)
    coff = np.broadcast_to(
        (np.arange(C) * H * W).astype(np.float32), (P, C)
    ).copy()
    vals = {
        "c_iota_w": iota_w,
        "c_iota_r": iota_r,
        "c_rev_r": rev_r,
        "c_y2": y2,
        "c_band": band,
        "c_bandrev": bandrev,
        "c_lo16": lo16,
        "c_hi16": hi16,
        "c_pp": pp,
        "c_2p": c2p,
        "c_coff": coff,
    }
    packed = np.zeros((P, C_TOTAL), dtype=np.float32)
    for kk, vv in vals.items():
        packed[:, _C_OFFS[kk] : _C_OFFS[kk] + _C_WIDTHS[kk]] = vv
    return {"c_all": packed}


def _build() -> bass.Bass:
    nc = bacc.Bacc("TRN2")
    x = nc.dram_tensor("x", [BPC, CH_IN, H, W], f32, kind="ExternalInput")
    y = nc.dram_tensor("y", [BPC, C, S, S], f32, kind="ExternalOutput")
    c_all = nc.dram_tensor("c_all", [P, C_TOTAL], f32, kind="ExternalInput")

    with TileContext(nc) as tc:
        with (
            tc.tile_pool(name="consts", bufs=1) as cpool,
            tc.tile_pool(name="work", bufs=4) as wp,
        ):
            call_t = cpool.tile([P, C_TOTAL], f32, tag="c_all", name="c_all_t")
            nc.sync.dma_start(call_t[:], c_all[:])
            ct = {
                k: call_t[:, _C_OFFS[k] : _C_OFFS[k] + _C_WIDTHS[k]]
                for k in _C_WIDTHS
            }
            # warm the Activation function table before sample work arrives
            warm = cpool.tile([P, 1], f32, tag="warm", name="warm")
            nc.scalar.activation(warm[:], call_t[:, 0:1], Act.Identity, scale=1.0)

            ts = nc.vector.tensor_scalar
            tsp = nc.gpsimd.tensor_scalar
            st = nc.vector.scalar_tensor_tensor
            tt = nc.vector.tensor_tensor
            ttp = nc.gpsimd.tensor_tensor
            red = nc.vector.tensor_reduce
            cpy = nc.vector.tensor_copy
            cpyp = nc.gpsimd.tensor_copy
            act = nc.scalar.activation

            for i in range(BPC):
                # ---- 1. coarse mask: rows 0,16,..,496 on partitions 0..31 ----
                mc = wp.tile([P, W], f32, tag="mc", name=f"mc{i}")
                xm = x[i, CH_IN - 1]
                nc.sync.dma_start(
                    mc[0:32, :],
                    bass.AP(xm.tensor, xm.offset, [[16 * W, 32], [1, W]]),
                )

                hp = tc.high_priority(offset=300)
                hp.__enter__()
                # per coarse row p: s~ = rowsum (sw+1 if in-window), m~ = moment
                cand = wp.tile([P, 4], f32, tag="cand", name=f"cand{i}")
                junkd = wp.tile([P, W], f32, tag="junkd", name=f"jd{i}")
                nc.vector.memset(cand[:], 0.0)
                red(cand[0:32, 2:3], mc[0:32, :], axis=Ax.X, op=Alu.add)  # s~
                st(junkd[0:32, :], mc[0:32, :], 1.0, ct["c_iota_w"][0:32, :],
                   op0=Alu.mult, op1=Alu.mult, accum_out=cand[0:32, 3:4])  # m~
                # band-any -> [k2cand, k1revcand]
                ra = wp.tile([P, 3], f32, tag="ra", name=f"ra{i}")
                ts(ra[0:32, 0:1], cand[0:32, 2:3], 0.0, None, op0=Alu.is_gt)
                tt(cand[0:32, 0:1], ra[0:32, 0:1], ct["c_band"][0:32, :],
                   op=Alu.mult)                           # k*any
                tt(cand[0:32, 1:2], ra[0:32, 0:1], ct["c_bandrev"][0:32, :],
                   op=Alu.mult)                           # (31-k)*any
                ar = wp.tile([P, 4], f32, tag="ar", name=f"ar{i}")
                nc.gpsimd.partition_all_reduce(
                    ar[:], cand[:], channels=P, reduce_op=bass_isa.ReduceOp.max
                )  # ar = [k2, 31-k1, s, m]

                # ---- 1b. refine: rows 16k1-15..16k1 and 16k2..16k2+15 ----
                # row(p) = clamp(lo16*(481-16*(31-k1)) + hi16*(16*k2-16) + p, 0)
                rr = wp.tile([P, 4], f32, tag="rr", name=f"rr{i}")
                ri2 = wp.tile([P, 1], i32, tag="ri2", name=f"ri2{i}")
                tsp(rr[:, 0:1], ar[:, 1:2], -16.0, 481.0, op0=Alu.mult, op1=Alu.add)
                tsp(rr[:, 1:2], ar[:, 0:1], 16.0, -16.0, op0=Alu.mult, op1=Alu.add)
                tsp(rr[:, 0:1], ct["c_lo16"], rr[:, 0:1], None, op0=Alu.mult)
                tsp(rr[:, 1:2], ct["c_hi16"], rr[:, 1:2], None, op0=Alu.mult)
                ttp(rr[:, 2:3], rr[:, 0:1], rr[:, 1:2], op=Alu.add)
                ttp(rr[:, 2:3], rr[:, 2:3], ct["c_pp"], op=Alu.add)
                tsp(rr[:, 2:3], rr[:, 2:3], 0.0, None, op0=Alu.max)  # row
                tsp(rr[:, 3:4], rr[:, 2:3], float(W),
                    float((CH_IN - 1) * H * W + i * CH_IN * H * W),
                    op0=Alu.mult, op1=Alu.add)            # flat idx
                cpyp(ri2[:], rr[:, 3:4])
                rf = wp.tile([P, W], f32, tag="rf", name=f"rf{i}")
                binst = nc.gpsimd.indirect_dma_start(
                    out=rf[0:32, :],
                    out_offset=None,
                    in_=bass.AP(x[:].tensor, 0, [[1, NEL - W + 1], [1, W]]),
                    in_offset=IndirectOffsetOnAxis(ap=ri2[0:32, :], axis=0),
                )
                a0 = binst.ins.ins[0]
                d0 = a0.dynamic_ap_info
                a0.dynamic_ap_info = mybir.DynamicAccessPatternInfo(
                    c=d0.c, actual_ap=d0.actual_ap,
                    indirect_dim_max_index=d0.indirect_dim_max_index,
                    offset_expr=[mybir.DynamicAccessPatternOffsetExpr(
                        coef=1, aff_expr=d0.offset_expr[0].aff_expr)])

                # refine row-any (full width) -> exact r0/r1 weighted maxes
                cand2 = wp.tile([P, 2], f32, tag="cand2", name=f"c2{i}")
                nc.vector.memset(cand2[:], 0.0)
                red(ra[0:32, 1:2], rf[0:32, :], axis=Ax.X, op=Alu.max)  # any
                # weights: p<16: (511-row)*any ; p>=16: row*any
                ts(ra[0:32, 2:3], rr[0:32, 2:3], -1.0, 511.0,
                   op0=Alu.mult, op1=Alu.add)
                tt(ra[0:32, 2:3], ra[0:32, 2:3], ct["c_lo16"][0:32, :],
                   op=Alu.mult)
                tt(cand2[0:32, 0:1], ra[0:32, 1:2], ra[0:32, 2:3], op=Alu.mult)
                ts(ra[0:32, 2:3], rr[0:32, 2:3], ct["c_hi16"][0:32, :], None,
                   op0=Alu.mult)
                tt(cand2[0:32, 1:2], ra[0:32, 1:2], ra[0:32, 2:3], op=Alu.mult)
                ar2 = wp.tile([P, 2], f32, tag="ar2", name=f"ar2{i}")
                nc.gpsimd.partition_all_reduce(
                    ar2[:], cand2[:], channels=P, reduce_op=bass_isa.ReduceOp.max
                )  # ar2 = [511-r0, r1]

                # ---- 4. scalar chain: recip on DVE, rest on Pool ----
                # ar = [r1, 511-r0, s~=sw+1, m~]
                # sc: 0 s1=r1+(511-r0), 1 rs, 2 q=2mu, 3 kf, 4 t_top,
                #     5 t_left, 6 top, 7 left, 8 c0, 9 rt, 10 d, 11 tph, 12 lw
                sc = wp.tile([P, 13], f32, tag="sc", name=f"sc{i}")
                sci = wp.tile([P, 3], i32, tag="sci", name=f"sci{i}")
                nc.vector.reciprocal(sc[:, 1:2], ar[:, 2:3])
                ts(sc[:, 0:1], ar2[:, 1:2], ar2[:, 0:1], None, op0=Alu.add)
                ts(sc[:, 2:3], ar[:, 3:4], sc[:, 1:2], 2.0,
                   op0=Alu.mult, op1=Alu.mult)           # 2*mu approx
                cpy(sci[:, 0:1], sc[:, 2:3])             # round -> k=c0+c1
                cpy(sc[:, 3:4], sci[:, 0:1])             # kf
                # floor((256-side)/2) via round(x*0.5-0.25):
                # t_top = (767-s1)/2-0.25; t_left = (257-s)/2-0.25
                ts(sc[:, 4:5], sc[:, 0:1], -0.5, 383.25, op0=Alu.mult, op1=Alu.add)
                ts(sc[:, 5:6], ar[:, 2:3], -0.5, 128.25, op0=Alu.mult, op1=Alu.add)
                cpy(sci[:, 1:3], sc[:, 4:6])             # round-to-nearest
                cpy(sc[:, 6:8], sci[:, 1:3])             # top, left
                ts(sc[:, 8:9], sc[:, 3:4], ar[:, 2:3], 1.0,
                   op0=Alu.subtract, op1=Alu.add)        # 2c0 = k - s + 1
                ts(sc[:, 8:9], sc[:, 8:9], 0.5, None, op0=Alu.mult)  # c0
                ts(sc[:, 9:10], ar2[:, 0:1], -1.0, 511.0,
                   op0=Alu.mult, op1=Alu.add)            # r0 (reuse col 9)
                tt(sc[:, 9:10], sc[:, 9:10], sc[:, 6:7], op=Alu.subtract)  # rt
                tt(sc[:, 10:11], sc[:, 8:9], sc[:, 7:8], op=Alu.subtract)  # d
                ts(sc[:, 11:12], sc[:, 6:7], sc[:, 0:1], -511.0,
                   op0=Alu.add, op1=Alu.add)             # tph = top+sh
                ts(sc[:, 12:13], sc[:, 7:8], ar[:, 2:3], -1.0,
                   op0=Alu.add, op1=Alu.add)             # lw = left+sw

                # ---- 5. gather indices + 3 indirect DMAs ----
                iy = wp.tile([P, 1], f32, tag="iy", name=f"iy{i}")
                ix = wp.tile([P, C], f32, tag="ix", name=f"ix{i}")
                ri = wp.tile([P, C], i32, tag="ri", name=f"ri{i}")
                ts(iy[:], ct["c_2p"], sc[:, 9:10], None, op0=Alu.add)
                ts(iy[:], iy[:], 0.0, float(H - 1), op0=Alu.max, op1=Alu.min)
                ts(iy[:], iy[:], float(W), None, op0=Alu.mult)
                ts(ix[:], ct["c_coff"], iy[:], float(i * CH_IN * H * W),
                   op0=Alu.add, op1=Alu.add)             # coff + y*512 + base
                ts(ix[:], ix[:], sc[:, 10:11], 0.0,
                   op0=Alu.add, op1=Alu.max)             # + d, clamp >= 0
                cpy(ri[:], ix[:])
                hp.__exit__(None, None, None)

                gat = wp.tile([P, C, GL], f32, tag="gat", name=f"gat{i}")
                for c in range(C):
                    # src viewed as overlapping GL-wide rows so descgen emits
                    # 128 x 3KB descriptors; coef patched to 1 for
                    # element-granular starts (row r, col d in one index).
                    binst = nc.gpsimd.indirect_dma_start(
                        out=gat[:, c, :],
                        out_offset=None,
                        in_=bass.AP(x[:].tensor, 0,
                                    [[1, NEL - GL + 1], [1, GL]]),
                        in_offset=IndirectOffsetOnAxis(ap=ri[:, c : c + 1], axis=0),
                    )
                    a0 = binst.ins.ins[0]
                    d0 = a0.dynamic_ap_info
                    a0.dynamic_ap_info = mybir.DynamicAccessPatternInfo(
                        c=d0.c, actual_ap=d0.actual_ap,
                        indirect_dim_max_index=d0.indirect_dim_max_index,
                        offset_expr=[mybir.DynamicAccessPatternOffsetExpr(
                            coef=1, aff_expr=d0.offset_expr[0].aff_expr)])

                # ---- 6. masks ----
                mya = wp.tile([P, 2], f32, tag="mya", name=f"mya{i}")
                myb = wp.tile([P, 2], f32, tag="myb", name=f"myb{i}")
                ts(mya[:], ct["c_y2"], sc[:, 6:7], None, op0=Alu.is_ge)
                ts(myb[:], ct["c_y2"], sc[:, 11:12], None, op0=Alu.is_lt)
                tt(mya[:], mya[:], myb[:], op=Alu.mult)  # my [128,2]
                mxa = wp.tile([P, S], f32, tag="mxa", name=f"mxa{i}")
                mxb = wp.tile([P, S], f32, tag="mxb", name=f"mxb{i}")
                ts(mxa[:], ct["c_iota_w"][:, :S], sc[:, 7:8], None, op0=Alu.is_ge)
                ts(mxb[:], ct["c_iota_w"][:, :S], sc[:, 12:13], None, op0=Alu.is_lt)
                tt(mxa[:], mxa[:], mxb[:], op=Alu.mult)   # mx [128,256]
                mk = wp.tile([P, 2, S], f32, tag="mk", name=f"mk{i}")
                for k in range(2):
                    act(mk[:, k, :], mxa[:], Act.Identity,
                        scale=mya[:, k : k + 1])

                # ---- 7. apply masks + store ----
                fin = wp.tile([P, C, 2, S], f32, tag="fin", name=f"fin{i}")
                yv = y[i]
                for c in range(C):
                    g = gat[:, c, :]
                    gv = bass.AP(g.tensor, g.offset,
                                 [list(g.ap[0]), [W, 2], [1, S]])
                    tt(fin[:, c], gv, mk[:], op=Alu.mult)
                    # y[i, c, 2p+k, x] <- fin[p, c, k, x]; (k x) merge to 512
                    ydst = bass.AP(yv.tensor, yv.offset + c * S * S,
                                   [[2 * S, P], [1, 2 * S]])
                    nc.sync.dma_start(ydst, fin[:, c].rearrange("p k x -> p (k x)"))
    nc.finalize()
    return nc


_CACHE: dict[str, object] = {}


def kernel(input1: np.ndarray, input2: np.ndarray, **_: np.ndarray) -> np.ndarray:
    input1 = np.ascontiguousarray(np.asarray(input1, dtype=np.float32))
    if "nc" not in _CACHE:
        _CACHE["nc"] = _build()
        _CACHE["consts"] = _consts()
    nc = _CACHE["nc"]
    consts = _CACHE["consts"]
    in_maps = [
        {"x": np.ascontiguousarray(input1[k * BPC : (k + 1) * BPC]), **consts}
        for k in range(N_CORES)
    ]
    res = run_bass_kernel_spmd(nc, in_maps, core_ids=list(range(N_CORES)))
    out = np.concatenate([r["y"] for r in res.results], axis=0)
    return out.astype(np.float32)


if __name__ == "__main__":
    rng = np.random.default_rng(1)
    x = rng.standard_normal((B, CH_IN, H, W), dtype=np.float32)
    print(kernel(x, np.zeros((B, C, S, S), np.float32)).shape)
